# revision 31
# baseline (speedup 1.0000x reference)
"""DCRNN (DCGRU encoder x8 + decoder x1 + projection) on 8 TRN2 NeuronCores.

Sharding: data-parallel over batch (B=64 -> 8 per core). The support matrix S
(symmetric scaled Laplacian, padded 1000->1024) and the GRU weights are
row/column-sharded on the wire (2.3MB total instead of 8x replicated) and
rebuilt on every core with a single on-device AllGather over NeuronLink.

Per-core on-device algorithm, per DCGRU cell:
  Z1 = S @ h, Z2 = S @ Z1          (node-major [n,(b,u)] PE matmuls, bf16)
  ru = sigmoid(h@A + Z1@B + Z2@C + x-part + bias)   (feature-major gate
       matmuls fed by PE DMA-transposes)
  rh = r*h; Z1' = S@rh; Z2' = S@Z1'
  c  = tanh(...); h = u*h + (1-u)*c                 (DVE elementwise)
Chebyshev recurrence + the f*K+k torch weight layout are folded on the host
into per-part weight blocks:  out = h@A + Z1@B + Z2@C + x*wx0 + Sx*wx1
+ S2x*wx2 + bias, with [A;B] (128,out) and [C;wx;bias] (68,out) stacks.

Dispatch: one persistent jax.jit(shard_map(bass_exec)) callable built once
per process (rebuilding it per call costs >1s of retracing); per call we ship
~5.5MB over the axon tunnel and fetch the (64,1024) output.
"""

import sys
import numpy as np

sys.path.insert(0, "/opt/trn_rl_repo")

from contextlib import ExitStack

import concourse.bass as bass
import concourse.bacc as bacc
import concourse.mybir as mybir
from concourse import tile

B, T, N, U = 64, 8, 1000, 64
NPAD = 1024
NCORES = 8
BC = B // NCORES          # 8 batch elements per core
NT = NPAD // 128          # 8 node tiles
FW = BC * U               # 512 free width: (b, u) b-major
DT = mybir.dt
AF = mybir.ActivationFunctionType

# packed weight SBUF tile [128, WPC]: column offsets of each block
WCOL = {
    "eA_ru": 0, "eB_ru": 128, "eA_c": 256, "eB_c": 320,
    "dA_ru": 384, "dB_ru": 512, "dA_c": 640, "dB_c": 704, "ident": 768,
}
WPC = 896
WSH = WPC // NCORES       # 112 wpack columns shipped per core
# S is symmetric: ship only upper-triangle 128x128 tiles (36, padded to 40
# slots so each core's column shard is 5 tile slots = 640 cols)
UPT = [(i, j) for i in range(NT) for j in range(i, NT)]
NSLOT = 40
SSH = NSLOT * 128 // NCORES   # 640 S-pack columns shipped per core
GINW = SSH + WSH              # 752: [S-triangle shard | wpack column shard]


def _prep_gate(W, b):
    """Fold Chebyshev recurrence + interleaved (f*K+k) weight layout into
    per-part blocks. out = x0@W0 + (S x0)@W1 + (2 S^2 x0 - x0)@W2 + b with
    x0 = [x | h]."""
    W = np.asarray(W, np.float32)
    b = np.asarray(b, np.float32)
    W0, W1, W2 = W[0::3], W[1::3], W[2::3]          # (65, out)
    A = W0[1:] - W2[1:]                             # h part
    Bh = W1[1:]                                     # Z1 part
    Ch = 2.0 * W2[1:]                               # Z2 part
    xrows = np.stack([W0[0] - W2[0], W1[0], 2.0 * W2[0]], 0)   # (3, out)
    blkA = np.concatenate([A, Bh], 0)               # (128, out)
    blkB = np.concatenate([Ch, xrows, b[None, :]], 0)  # (68, out)
    return blkA, blkB


def _build_program():
    nc = bacc.Bacc(None)

    # one bf16 input blob per core: [gather shard (S rows | wpack cols) | x]
    # x part is node-major: gx[p, GINW + j*64 + b*8 + t] = x[b, t, j*128+p]
    XW = NT * T * BC
    dGx = nc.declare_dram_parameter("gx", [128, GINW + XW], DT.bfloat16, False)
    dWp = nc.declare_dram_parameter("wp", [1, FW], DT.float32, False)
    dOut = nc.declare_dram_parameter("out", [BC, NPAD], DT.float32, True)

    with ExitStack() as ctx:
        tc = ctx.enter_context(tile.TileContext(nc))
        dram = ctx.enter_context(tc.tile_pool(name="dram", bufs=1, space="DRAM"))
        const = ctx.enter_context(tc.tile_pool(name="const", bufs=1))
        state = ctx.enter_context(tc.tile_pool(name="state", bufs=1))
        psS = ctx.enter_context(tc.tile_pool(name="psS", bufs=2, space="PSUM"))
        psG = ctx.enter_context(tc.tile_pool(name="psG", bufs=2, space="PSUM"))
        psT = ctx.enter_context(tc.tile_pool(name="psT", bufs=4, space="PSUM"))
        tmpp = ctx.enter_context(tc.tile_pool(name="tmpp", bufs=3))

        # --- gather S + packed weights from all cores -------------------------
        g_in_b = dram.tile([128, GINW], DT.bfloat16, tag="g_in_b")
        g_out = dram.tile([NCORES * 128, GINW], DT.bfloat16, tag="g_out",
                          addr_space="Shared")
        nc.gpsimd.dma_start(out=g_in_b[:], in_=dGx[:, 0:GINW])
        nc.gpsimd.collective_compute(
            "AllGather",
            mybir.AluOpType.bypass,
            replica_groups=[list(range(NCORES))],
            ins=[g_in_b.opt()],
            outs=[g_out.opt()],
        )

        # --- resident tensors -------------------------------------------------
        # S_sb[p, (i*NT+j)*128+q] = S_pad[i*128+p, j*128+q]; upper tiles come
        # from the gather, lower tiles by PE-transposing their mirror (S = S^T)
        wpk = const.tile([128, WPC], DT.bfloat16, tag="wpk")
        for c in range(NCORES):
            nc.sync.dma_start(
                out=wpk[:, c * WSH:(c + 1) * WSH],
                in_=g_out[c * 128:(c + 1) * 128, SSH:GINW],
            )
        ident = wpk[:, WCOL["ident"]:WCOL["ident"] + 128]

        S_sb = const.tile([128, NT * NT * 128], DT.bfloat16, tag="S_sb")
        for s, (i, j) in enumerate(UPT):
            nc.sync.dma_start(
                out=S_sb[:, (i * NT + j) * 128:(i * NT + j + 1) * 128],
                in_=g_out[(s // 5) * 128:(s // 5 + 1) * 128,
                          (s % 5) * 128:(s % 5 + 1) * 128],
            )
        for i, j in UPT:
            if i == j:
                continue
            pt = psT.tile([128, 128], DT.bfloat16, tag="pt")
            nc.tensor.transpose(
                pt[:], S_sb[:, (i * NT + j) * 128:(i * NT + j + 1) * 128],
                ident)
            nc.scalar.copy(
                S_sb[:, (j * NT + i) * 128:(j * NT + i + 1) * 128], pt[:])

        # broadcast wp row to all 128 partitions via a rank-1 PE outer product
        wp_row = const.tile([1, FW], DT.float32, tag="wp_row")
        ones_t = const.tile([1, 128], DT.float32, tag="ones_t")
        wp_sb = const.tile([128, FW], DT.float32, tag="wp_sb")
        nc.sync.dma_start(out=wp_row[0:1, :], in_=dWp[:])
        nc.vector.memset(ones_t[0:1, :], 1.0)
        ps_w = psS.tile([128, FW], DT.float32, tag="psS")
        nc.tensor.matmul(ps_w[:], lhsT=ones_t[0:1, :], rhs=wp_row[0:1, :],
                         start=True, stop=True)
        nc.vector.tensor_copy(wp_sb[:], ps_w[:])

        Gfa = state.tile([128, BC * NPAD], DT.bfloat16, tag="Gfa")
        Gfb = state.tile([128, BC * NPAD], DT.bfloat16, tag="Gfb")
        h = state.tile([128, NT * FW], DT.float32, tag="h")
        hbf = state.tile([128, NT * FW], DT.bfloat16, tag="hbf")
        z1bf = state.tile([128, NT * FW], DT.bfloat16, tag="z1bf")
        z2bf = state.tile([128, NT * FW], DT.bfloat16, tag="z2bf")
        rhbf = state.tile([128, NT * FW], DT.bfloat16, tag="rhbf")
        r_s = state.tile([128, NT * FW], DT.float32, tag="r_s")   # r, then rh
        u_s = state.tile([128, NT * FW], DT.float32, tag="u_s")
        c_s = state.tile([128, NT * FW], DT.float32, tag="c_s")
        out_sb = state.tile([128, NT * BC], DT.float32, tag="out_sb")

        nc.vector.memset(h[:], 0.0)
        nc.vector.memset(hbf[:], 0.0)
        nc.vector.memset(Gfa[:], 0.0)
        nc.vector.memset(Gfb[0:64, :], 0.0)

        def gfa_fill(src0_bf, src1_bf):
            # PE-transpose src0 (rows 0:64) + src1 (rows 64:128) per (j,b)
            # into one PSUM tile, one ACT copy out to Gfa.
            for j in range(NT):
                for b in range(BC):
                    pt = psT.tile([128, 128], DT.bfloat16, tag="pt")
                    s = slice(j * FW + b * 64, j * FW + (b + 1) * 64)
                    nc.tensor.transpose(pt[0:64, :], src0_bf[:, s], ident)
                    nc.tensor.transpose(pt[64:128, :], src1_bf[:, s], ident)
                    col = b * NPAD + j * 128
                    nc.scalar.copy(Gfa[:, col:col + 128], pt[:])

        def gfb_fill(src_bf):
            for j in range(NT):
                for b in range(BC):
                    pt = psT.tile([128, 128], DT.bfloat16, tag="pt")
                    s = slice(j * FW + b * 64, j * FW + (b + 1) * 64)
                    nc.tensor.transpose(pt[0:64, :], src_bf[:, s], ident)
                    col = b * NPAD + j * 128
                    nc.scalar.copy(Gfb[0:64, col:col + 128], pt[0:64, :])

        def smatmul(rhs_bf, out_bf):
            # Z = S @ rhs  (node-major in/out), bf16 on PE, fp32 accum
            for j in range(NT):
                ps = psS.tile([128, FW], DT.float32, tag="psS")
                for i in range(NT):
                    nc.tensor.matmul(
                        ps[:],
                        lhsT=S_sb[:, (i * NT + j) * 128:(i * NT + j + 1) * 128],
                        rhs=rhs_bf[:, i * FW:(i + 1) * FW],
                        start=(i == 0),
                        stop=(i == NT - 1),
                    )
                nc.vector.tensor_copy(out_bf[:, j * FW:(j + 1) * FW], ps[:])

        # --- x features: x, Sx, S2x computed on device ------------------------
        # node-major x tiles [128, j*64 + t*8 + b], diffused by S on the PE,
        # then DMA-scattered into DRAM in feature-major per-cell row layout
        # dxf_k[t, b*NPAD + j*128 + p]  (rows T: zeros for GO, T+1 on k=0: ones)
        FWX = T * BC
        x0_nm = state.tile([128, NT * FWX], DT.bfloat16, tag="x0_nm")
        x1_nm = state.tile([128, NT * FWX], DT.bfloat16, tag="x1_nm")
        x2_nm = state.tile([128, NT * FWX], DT.bfloat16, tag="x2_nm")
        nc.sync.dma_start(out=x0_nm[:], in_=dGx[:, GINW:GINW + XW])

        def smatmul_x(rhs_bf, out_bf):
            for j in range(NT):
                ps = psS.tile([128, FW], DT.float32, tag="psS")
                for i in range(NT):
                    nc.tensor.matmul(
                        ps[:, 0:FWX],
                        lhsT=S_sb[:, (i * NT + j) * 128:(i * NT + j + 1) * 128],
                        rhs=rhs_bf[:, i * FWX:(i + 1) * FWX],
                        start=(i == 0),
                        stop=(i == NT - 1),
                    )
                nc.vector.tensor_copy(out_bf[:, j * FWX:(j + 1) * FWX],
                                      ps[:, 0:FWX])

        smatmul_x(x0_nm, x1_nm)
        smatmul_x(x1_nm, x2_nm)

        zrow = const.tile([1, BC * NPAD], DT.bfloat16, tag="zrow")
        orow = const.tile([1, BC * NPAD], DT.bfloat16, tag="orow")
        nc.vector.memset(zrow[0:1, :], 0.0)
        nc.vector.memset(orow[0:1, :], 1.0)
        dxf = []
        for k, src in ((0, x0_nm), (1, x1_nm), (2, x2_nm)):
            dx = dram.tile([T + 2, BC * NPAD], DT.bfloat16, tag=f"dxf{k}",
                           name=f"dxf{k}")
            for j in range(NT):
                # [128 nodes, (b t)] -> PE transpose -> [(b t), 128 nodes]
                pt = psT.tile([128, 128], DT.bfloat16, tag="pt")
                nc.tensor.transpose(pt[0:64, :],
                                    src[:, j * FWX:(j + 1) * FWX], ident)
                xt = tmpp.tile([128, 128], DT.bfloat16, tag="xt")
                nc.scalar.copy(xt[0:64, :], pt[0:64, :])
                for b in range(BC):
                    o = b * NPAD + j * 128
                    nc.sync.dma_start(out=dx[0:T, o:o + 128],
                                      in_=xt[b * T:(b + 1) * T, :])
            nc.sync.dma_start(out=dx[T:T + 1, :], in_=zrow[0:1, :])
            dxf.append(dx)
        nc.sync.dma_start(out=dxf[0][T + 1:T + 2, :], in_=orow[0:1, :])
        nc.sync.dma_start(out=Gfb[67:68, :], in_=dxf[0][T + 1:T + 2, :])

        def gates(wa_off, wb_off, width, fn, dst0, dst1):
            # psum[m,out] = Gfa_slice.T @ wa + Gfb_slice.T @ wb ; act -> dst
            for j in range(NT):
                for b in range(BC):
                    pg = psG.tile([128, 128], DT.float32, tag="psG")
                    col = b * NPAD + j * 128
                    nc.tensor.matmul(
                        pg[:, 0:width], lhsT=Gfa[:, col:col + 128],
                        rhs=wpk[:, wa_off:wa_off + width], start=True, stop=False,
                    )
                    nc.tensor.matmul(
                        pg[:, 0:width], lhsT=Gfb[0:68, col:col + 128],
                        rhs=wpk[0:68, wb_off:wb_off + width], start=False, stop=True,
                    )
                    o = j * FW + b * 64
                    if width == 128:
                        nc.scalar.activation(dst0[:, o:o + 64], pg[:, 0:64], fn)
                        nc.scalar.activation(dst1[:, o:o + 64], pg[:, 64:128], fn)
                    else:
                        nc.scalar.activation(dst0[:, o:o + 64], pg[:, 0:64], fn)

        # --- the 9 DCGRU cells ------------------------------------------------
        for t in range(T + 1):
            enc = t < T
            wa_ru = WCOL["eA_ru" if enc else "dA_ru"]
            wb_ru = WCOL["eB_ru" if enc else "dB_ru"]
            wa_c = WCOL["eA_c" if enc else "dA_c"]
            wb_c = WCOL["eB_c" if enc else "dB_c"]

            if t > 0:  # cell 0: h == 0, so Z1 = Z2 = 0 and Gfa/Gfb
                smatmul(hbf, z1bf)                 # Z1 = S h
                gfa_fill(hbf, z1bf)                # h | Z1 features
                smatmul(z1bf, z2bf)                # Z2 = S Z1
                gfb_fill(z2bf)                     # Z2 features
            # t == T: the GO symbol x = 0 comes from the zero rows of dxf
            for k in range(3):
                nc.sync.dma_start(out=Gfb[64 + k:65 + k, :],
                                  in_=dxf[k][t:t + 1, :])

            gates(wa_ru, wb_ru, 128, AF.Sigmoid, r_s, u_s)

            for j in range(NT):
                js = slice(j * FW, (j + 1) * FW)
                nc.vector.tensor_mul(r_s[:, js], r_s[:, js], h[:, js])  # rh
                nc.scalar.copy(rhbf[:, js], r_s[:, js])                 # rh bf16
            if t > 0:  # cell 0: rh = r*0 = 0, Z1' = Z2' = 0
                smatmul(rhbf, z1bf)                # Z1' = S rh
                gfa_fill(rhbf, z1bf)               # rh | Z1' features
                smatmul(z1bf, z2bf)                # Z2' = S Z1'
                gfb_fill(z2bf)

            gates(wa_c, wb_c, 64, AF.Tanh, c_s, None)

            for j in range(NT):
                js = slice(j * FW, (j + 1) * FW)
                tmp = tmpp.tile([128, FW], DT.float32, tag="tmp")
                nc.vector.tensor_sub(tmp[:], h[:, js], c_s[:, js])
                nc.vector.tensor_mul(tmp[:], tmp[:], u_s[:, js])
                nc.vector.tensor_add(h[:, js], c_s[:, js], tmp[:])
                nc.scalar.copy(hbf[:, js], h[:, js])

        # --- projection: out[b, m] = sum_u h * Wp + bp ------------------------
        for j in range(NT):
            js = slice(j * FW, (j + 1) * FW)
            tmp = tmpp.tile([128, FW], DT.float32, tag="tmp")
            nc.vector.tensor_mul(tmp[:], h[:, js], wp_sb[:])
            for b in range(BC):
                nc.vector.reduce_sum(
                    out_sb[:, j * BC + b:j * BC + b + 1],
                    tmp[:, b * 64:(b + 1) * 64],
                    axis=mybir.AxisListType.X,
                )
        for j in range(NT):
            nc.sync.dma_start(
                out=dOut[:, j * 128:(j + 1) * 128].rearrange("b p -> p b"),
                in_=out_sb[:, j * BC:(j + 1) * BC],
            )
    nc.finalize()
    return nc


_RUNNER = None  # (sharded_fn, in_names, out_names, out_shapes) — program-static
_RUNNER_MESH = None


def _get_runner():
    global _RUNNER
    if _RUNNER is not None:
        return _RUNNER

    import jax
    from jax.sharding import Mesh, PartitionSpec
    from jax.experimental.shard_map import shard_map
    from concourse.bass2jax import (
        _bass_exec_p, partition_id_tensor, install_neuronx_cc_hook,
    )

    nc = _build_program()
    install_neuronx_cc_hook()

    partition_name = (
        nc.partition_id_tensor.name if nc.partition_id_tensor else None
    )
    in_names, out_names, out_avals, out_shapes = [], [], [], []
    for alloc in nc.m.functions[0].allocations:
        if not isinstance(alloc, mybir.MemoryLocationSet):
            continue
        name = alloc.memorylocations[0].name
        if alloc.kind == "ExternalInput":
            if name != partition_name:
                in_names.append(name)
        elif alloc.kind == "ExternalOutput":
            shape = tuple(alloc.tensor_shape)
            dtype = mybir.dt.np(alloc.dtype)
            out_names.append(name)
            out_avals.append(jax.core.ShapedArray(shape, dtype))
            out_shapes.append((shape, dtype))
    n_params = len(in_names)
    n_outs = len(out_names)
    in_names_full = list(in_names) + out_names
    if partition_name is not None:
        in_names_full.append(partition_name)
    donate = tuple(range(n_params, n_params + n_outs))

    def _body(*args):
        operands = list(args)
        if partition_name is not None:
            operands.append(partition_id_tensor())
        outs = _bass_exec_p.bind(
            *operands,
            out_avals=tuple(out_avals),
            in_names=tuple(in_names_full),
            out_names=tuple(out_names),
            lowering_input_output_aliases=(),
            sim_require_finite=True,
            sim_require_nnan=True,
            nc=nc,
        )
        return tuple(outs)

    global _RUNNER_MESH
    devices = jax.devices()[:NCORES]
    mesh = Mesh(np.asarray(devices), ("core",))
    _RUNNER_MESH = mesh
    in_specs = (PartitionSpec("core"),) * (n_params + n_outs)
    out_specs = (PartitionSpec("core"),) * n_outs
    sharded = jax.jit(
        shard_map(_body, mesh=mesh, in_specs=in_specs, out_specs=out_specs,
                  check_rep=False),
        donate_argnums=donate,
        keep_unused=True,
    )
    _RUNNER = (sharded, in_names, out_names, out_shapes)
    return _RUNNER


def kernel(inputs, support, enc_W_ru, enc_b_ru, enc_W_c, enc_b_c,
           dec_W_ru, dec_b_ru, dec_W_c, dec_b_c, W_proj, b_proj):
    import ml_dtypes
    bf16 = ml_dtypes.bfloat16

    inputs = np.asarray(inputs, np.float32)
    support = np.asarray(support, np.float32)
    W_proj = np.asarray(W_proj, np.float32)
    b_proj = np.asarray(b_proj, np.float32)

    # gin: [S row shard | wpack column shard], gathered on device
    wpk_full = np.zeros((128, WPC), np.float32)
    for nm, (blkA, blkB) in (
        ("e_ru", _prep_gate(enc_W_ru, enc_b_ru)),
        ("e_c", _prep_gate(enc_W_c, enc_b_c)),
        ("d_ru", _prep_gate(dec_W_ru, dec_b_ru)),
        ("d_c", _prep_gate(dec_W_c, dec_b_c)),
    ):
        pre = nm[0]
        suf = nm[2:]
        oa = WCOL[f"{pre}A_{suf}"]
        ob = WCOL[f"{pre}B_{suf}"]
        wpk_full[:, oa:oa + blkA.shape[1]] = blkA
        wpk_full[0:68, ob:ob + blkB.shape[1]] = blkB
    wpk_full[:, WCOL["ident"]:WCOL["ident"] + 128] = np.eye(128)

    XW = NT * T * BC
    gx = np.empty((NCORES * 128, GINW + XW), bf16)
    S_pad = np.zeros((NPAD, NPAD), np.float32)
    S_pad[:N, :N] = support
    spack = np.zeros((128, NSLOT * 128), np.float32)
    for s, (i, j) in enumerate(UPT):
        spack[:, s * 128:(s + 1) * 128] = S_pad[i * 128:(i + 1) * 128,
                                                j * 128:(j + 1) * 128]
    gx[:, :SSH] = (
        spack.astype(bf16).reshape(128, NCORES, SSH)
        .transpose(1, 0, 2).reshape(NCORES * 128, SSH)
    )
    gx[:, SSH:GINW] = (
        wpk_full.astype(bf16).reshape(128, NCORES, WSH)
        .transpose(1, 0, 2).reshape(NCORES * 128, WSH)
    )
    # node-major x: gx[c*128+p, GINW + j*64 + b*8 + t] = x[c*8+b, t, j*128+p]
    xp = np.zeros((B, T, NPAD), np.float32)
    xp[:, :, :N] = inputs
    gx[:, GINW:] = (
        xp.reshape(NCORES, BC, T, NT, 128).transpose(0, 4, 3, 1, 2)
        .reshape(NCORES * 128, XW).astype(bf16)
    )

    wpg = np.broadcast_to(
        np.tile(W_proj[:, 0].astype(np.float32), BC)[None, :], (NCORES, FW)
    )

    sharded, in_names, out_names, out_shapes = _get_runner()
    host = {"gx": gx, "wp": np.ascontiguousarray(wpg)}
    args = [host[nm] for nm in in_names]
    # The kernel writes every element of each output, so the donated output
    # canvases' contents are irrelevant: recycle the previous call's output
    # device buffers instead of uploading fresh zeros. The first call ships
    # zeros as committed device arrays so the jit signature never changes.
    global _PREV_OUTS
    if _PREV_OUTS is None:
        import jax
        from jax.sharding import NamedSharding, PartitionSpec
        mesh = _RUNNER_MESH
        sh = NamedSharding(mesh, PartitionSpec("core"))
        canvases = [
            jax.device_put(
                np.zeros((NCORES * shape[0], *shape[1:]), dtype), sh)
            for shape, dtype in out_shapes
        ]
    else:
        canvases = _PREV_OUTS
    outs = sharded(*args, *canvases)
    res = np.asarray(outs[out_names.index("out")])
    _PREV_OUTS = list(outs)
    return res[:, :N] + b_proj[0]


_PREV_OUTS = None


if __name__ == "__main__":
    pass


# revision 34
# speedup vs baseline: 1.0926x; 1.0926x over previous
"""DCRNN (DCGRU encoder x8 + decoder x1 + projection) on 8 TRN2 NeuronCores.

Sharding: data-parallel over batch (B=64 -> 8 per core). The support matrix S
(symmetric scaled Laplacian, padded 1000->1024) and the GRU weights are
row/column-sharded on the wire (2.3MB total instead of 8x replicated) and
rebuilt on every core with a single on-device AllGather over NeuronLink.

Per-core on-device algorithm, per DCGRU cell:
  Z1 = S @ h, Z2 = S @ Z1          (node-major [n,(b,u)] PE matmuls, bf16)
  ru = sigmoid(h@A + Z1@B + Z2@C + x-part + bias)   (feature-major gate
       matmuls fed by PE DMA-transposes)
  rh = r*h; Z1' = S@rh; Z2' = S@Z1'
  c  = tanh(...); h = u*h + (1-u)*c                 (DVE elementwise)
Chebyshev recurrence + the f*K+k torch weight layout are folded on the host
into per-part weight blocks:  out = h@A + Z1@B + Z2@C + x*wx0 + Sx*wx1
+ S2x*wx2 + bias, with [A;B] (128,out) and [C;wx;bias] (68,out) stacks.

Dispatch: one persistent jax.jit(shard_map(bass_exec)) callable built once
per process (rebuilding it per call costs >1s of retracing); per call we ship
~5.5MB over the axon tunnel and fetch the (64,1024) output.
"""

import sys
import numpy as np

sys.path.insert(0, "/opt/trn_rl_repo")

from contextlib import ExitStack

import concourse.bass as bass
import concourse.bacc as bacc
import concourse.mybir as mybir
from concourse import tile

B, T, N, U = 64, 8, 1000, 64
NPAD = 1024
NCORES = 8
BC = B // NCORES          # 8 batch elements per core
NT = NPAD // 128          # 8 node tiles
FW = BC * U               # 512 free width: (b, u) b-major
DT = mybir.dt
AF = mybir.ActivationFunctionType

# packed weight SBUF tile [128, WPC]: column offsets of each block
WCOL = {
    "eA_ru": 0, "eB_ru": 128, "eA_c": 256, "eB_c": 320,
    "dA_ru": 384, "dB_ru": 512, "dA_c": 640, "dB_c": 704, "ident": 768,
}
WPC = 896
WSH = WPC // NCORES       # 112 wpack columns shipped per core
# S is symmetric: ship only upper-triangle 128x128 tiles (36, padded to 40
# slots so each core's column shard is 5 tile slots = 640 cols)
UPT = [(i, j) for i in range(NT) for j in range(i, NT)]
NSLOT = 40
SSH = NSLOT * 128 // NCORES   # 640 S-pack columns shipped per core
GINW = SSH + WSH              # 752: [S-triangle shard | wpack column shard]


def _prep_gate(W, b):
    """Fold Chebyshev recurrence + interleaved (f*K+k) weight layout into
    per-part blocks. out = x0@W0 + (S x0)@W1 + (2 S^2 x0 - x0)@W2 + b with
    x0 = [x | h]."""
    W = np.asarray(W, np.float32)
    b = np.asarray(b, np.float32)
    W0, W1, W2 = W[0::3], W[1::3], W[2::3]          # (65, out)
    A = W0[1:] - W2[1:]                             # h part
    Bh = W1[1:]                                     # Z1 part
    Ch = 2.0 * W2[1:]                               # Z2 part
    xrows = np.stack([W0[0] - W2[0], W1[0], 2.0 * W2[0]], 0)   # (3, out)
    blkA = np.concatenate([A, Bh], 0)               # (128, out)
    blkB = np.concatenate([Ch, xrows, b[None, :]], 0)  # (68, out)
    return blkA, blkB


def _build_program():
    nc = bacc.Bacc(None)

    # one bf16 input blob per core: [gather shard (S rows | wpack cols) | x]
    # x part is node-major: gx[p, GINW + j*64 + b*8 + t] = x[b, t, j*128+p]
    XW = NT * T * BC
    dGx = nc.declare_dram_parameter("gx", [128, GINW + XW], DT.bfloat16, False)
    dWp = nc.declare_dram_parameter("wp", [1, FW], DT.float32, False)
    dOut = nc.declare_dram_parameter("out", [BC, NPAD], DT.float32, True)

    with ExitStack() as ctx:
        tc = ctx.enter_context(tile.TileContext(nc))
        dram = ctx.enter_context(tc.tile_pool(name="dram", bufs=1, space="DRAM"))
        const = ctx.enter_context(tc.tile_pool(name="const", bufs=1))
        state = ctx.enter_context(tc.tile_pool(name="state", bufs=1))
        psS = ctx.enter_context(tc.tile_pool(name="psS", bufs=2, space="PSUM"))
        psG = ctx.enter_context(tc.tile_pool(name="psG", bufs=2, space="PSUM"))
        psT = ctx.enter_context(tc.tile_pool(name="psT", bufs=4, space="PSUM"))
        tmpp = ctx.enter_context(tc.tile_pool(name="tmpp", bufs=3))

        # --- gather S + packed weights from all cores -------------------------
        g_in_b = dram.tile([128, GINW], DT.bfloat16, tag="g_in_b")
        g_out = dram.tile([NCORES * 128, GINW], DT.bfloat16, tag="g_out",
                          addr_space="Shared")
        nc.gpsimd.dma_start(out=g_in_b[:], in_=dGx[:, 0:GINW])
        nc.gpsimd.collective_compute(
            "AllGather",
            mybir.AluOpType.bypass,
            replica_groups=[list(range(NCORES))],
            ins=[g_in_b.opt()],
            outs=[g_out.opt()],
        )

        # --- resident tensors -------------------------------------------------
        # S_sb[p, (i*NT+j)*128+q] = S_pad[i*128+p, j*128+q]; upper tiles come
        # from the gather, lower tiles by PE-transposing their mirror (S = S^T)
        wpk = const.tile([128, WPC], DT.bfloat16, tag="wpk")
        for c in range(NCORES):
            nc.sync.dma_start(
                out=wpk[:, c * WSH:(c + 1) * WSH],
                in_=g_out[c * 128:(c + 1) * 128, SSH:GINW],
            )
        ident = wpk[:, WCOL["ident"]:WCOL["ident"] + 128]

        S_sb = const.tile([128, NT * NT * 128], DT.bfloat16, tag="S_sb")
        for s, (i, j) in enumerate(UPT):
            nc.sync.dma_start(
                out=S_sb[:, (i * NT + j) * 128:(i * NT + j + 1) * 128],
                in_=g_out[(s // 5) * 128:(s // 5 + 1) * 128,
                          (s % 5) * 128:(s % 5 + 1) * 128],
            )
        for i, j in UPT:
            if i == j:
                continue
            pt = psT.tile([128, 128], DT.bfloat16, tag="pt")
            nc.tensor.transpose(
                pt[:], S_sb[:, (i * NT + j) * 128:(i * NT + j + 1) * 128],
                ident)
            nc.scalar.copy(
                S_sb[:, (j * NT + i) * 128:(j * NT + i + 1) * 128], pt[:])

        # broadcast wp row to all 128 partitions via a rank-1 PE outer product
        wp_row = const.tile([1, FW], DT.float32, tag="wp_row")
        ones_t = const.tile([1, 128], DT.float32, tag="ones_t")
        wp_sb = const.tile([128, FW], DT.float32, tag="wp_sb")
        nc.sync.dma_start(out=wp_row[0:1, :], in_=dWp[:])
        nc.vector.memset(ones_t[0:1, :], 1.0)
        ps_w = psS.tile([128, FW], DT.float32, tag="psS")
        nc.tensor.matmul(ps_w[:], lhsT=ones_t[0:1, :], rhs=wp_row[0:1, :],
                         start=True, stop=True)
        nc.vector.tensor_copy(wp_sb[:], ps_w[:])

        Gfa = state.tile([128, BC * NPAD], DT.bfloat16, tag="Gfa")
        Gfb = state.tile([128, BC * NPAD], DT.bfloat16, tag="Gfb")
        h = state.tile([128, NT * FW], DT.float32, tag="h")
        hbf = state.tile([128, NT * FW], DT.bfloat16, tag="hbf")
        z1bf = state.tile([128, NT * FW], DT.bfloat16, tag="z1bf")
        z2bf = state.tile([128, NT * FW], DT.bfloat16, tag="z2bf")
        rhbf = state.tile([128, NT * FW], DT.bfloat16, tag="rhbf")
        r_s = state.tile([128, NT * FW], DT.float32, tag="r_s")   # r, then rh
        u_s = state.tile([128, NT * FW], DT.float32, tag="u_s")
        c_s = state.tile([128, NT * FW], DT.float32, tag="c_s")
        out_sb = state.tile([128, NT * BC], DT.float32, tag="out_sb")

        nc.vector.memset(h[:], 0.0)
        nc.vector.memset(hbf[:], 0.0)
        nc.vector.memset(Gfa[:], 0.0)
        nc.vector.memset(Gfb[0:64, :], 0.0)

        def gfa_fill(src0_bf, src1_bf):
            # PE-transpose src0 (rows 0:64) + src1 (rows 64:128) per (j,b)
            # into one PSUM tile, one ACT copy out to Gfa.
            for j in range(NT):
                for b in range(BC):
                    pt = psT.tile([128, 128], DT.bfloat16, tag="pt")
                    s = slice(j * FW + b * 64, j * FW + (b + 1) * 64)
                    nc.tensor.transpose(pt[0:64, :], src0_bf[:, s], ident)
                    nc.tensor.transpose(pt[64:128, :], src1_bf[:, s], ident)
                    col = b * NPAD + j * 128
                    nc.scalar.copy(Gfa[:, col:col + 128], pt[:])

        def gfb_fill(src_bf):
            for j in range(NT):
                for b in range(BC):
                    pt = psT.tile([128, 128], DT.bfloat16, tag="pt")
                    s = slice(j * FW + b * 64, j * FW + (b + 1) * 64)
                    nc.tensor.transpose(pt[0:64, :], src_bf[:, s], ident)
                    col = b * NPAD + j * 128
                    nc.scalar.copy(Gfb[0:64, col:col + 128], pt[0:64, :])

        def smatmul(rhs_bf, out_bf):
            # Z = S @ rhs  (node-major in/out), bf16 on PE, fp32 accum
            for j in range(NT):
                ps = psS.tile([128, FW], DT.float32, tag="psS")
                for i in range(NT):
                    nc.tensor.matmul(
                        ps[:],
                        lhsT=S_sb[:, (i * NT + j) * 128:(i * NT + j + 1) * 128],
                        rhs=rhs_bf[:, i * FW:(i + 1) * FW],
                        start=(i == 0),
                        stop=(i == NT - 1),
                    )
                nc.vector.tensor_copy(out_bf[:, j * FW:(j + 1) * FW], ps[:])

        # --- x features: x, Sx, S2x computed on device ------------------------
        # node-major x tiles [128, j*64 + t*8 + b], diffused by S on the PE,
        # then DMA-scattered into DRAM in feature-major per-cell row layout
        # dxf_k[t, b*NPAD + j*128 + p]  (rows T: zeros for GO, T+1 on k=0: ones)
        FWX = T * BC
        x0_nm = state.tile([128, NT * FWX], DT.bfloat16, tag="x0_nm")
        x1_nm = state.tile([128, NT * FWX], DT.bfloat16, tag="x1_nm")
        x2_nm = state.tile([128, NT * FWX], DT.bfloat16, tag="x2_nm")
        nc.sync.dma_start(out=x0_nm[:], in_=dGx[:, GINW:GINW + XW])

        def smatmul_x(rhs_bf, out_bf):
            for j in range(NT):
                ps = psS.tile([128, FW], DT.float32, tag="psS")
                for i in range(NT):
                    nc.tensor.matmul(
                        ps[:, 0:FWX],
                        lhsT=S_sb[:, (i * NT + j) * 128:(i * NT + j + 1) * 128],
                        rhs=rhs_bf[:, i * FWX:(i + 1) * FWX],
                        start=(i == 0),
                        stop=(i == NT - 1),
                    )
                nc.vector.tensor_copy(out_bf[:, j * FWX:(j + 1) * FWX],
                                      ps[:, 0:FWX])

        smatmul_x(x0_nm, x1_nm)
        smatmul_x(x1_nm, x2_nm)

        zrow = const.tile([1, BC * NPAD], DT.bfloat16, tag="zrow")
        orow = const.tile([1, BC * NPAD], DT.bfloat16, tag="orow")
        nc.vector.memset(zrow[0:1, :], 0.0)
        nc.vector.memset(orow[0:1, :], 1.0)
        dxf = []
        for k, src in ((0, x0_nm), (1, x1_nm), (2, x2_nm)):
            dx = dram.tile([T + 2, BC * NPAD], DT.bfloat16, tag=f"dxf{k}",
                           name=f"dxf{k}")
            for j in range(NT):
                # [128 nodes, (b t)] -> PE transpose -> [(b t), 128 nodes]
                pt = psT.tile([128, 128], DT.bfloat16, tag="pt")
                nc.tensor.transpose(pt[0:64, :],
                                    src[:, j * FWX:(j + 1) * FWX], ident)
                xt = tmpp.tile([128, 128], DT.bfloat16, tag="xt")
                nc.scalar.copy(xt[0:64, :], pt[0:64, :])
                for b in range(BC):
                    o = b * NPAD + j * 128
                    nc.sync.dma_start(out=dx[0:T, o:o + 128],
                                      in_=xt[b * T:(b + 1) * T, :])
            nc.sync.dma_start(out=dx[T:T + 1, :], in_=zrow[0:1, :])
            dxf.append(dx)
        nc.sync.dma_start(out=dxf[0][T + 1:T + 2, :], in_=orow[0:1, :])
        nc.sync.dma_start(out=Gfb[67:68, :], in_=dxf[0][T + 1:T + 2, :])

        def gates(wa_off, wb_off, width, fn, dst0, dst1):
            # psum[m,out] = Gfa_slice.T @ wa + Gfb_slice.T @ wb ; act -> dst
            for j in range(NT):
                for b in range(BC):
                    pg = psG.tile([128, 128], DT.float32, tag="psG")
                    col = b * NPAD + j * 128
                    nc.tensor.matmul(
                        pg[:, 0:width], lhsT=Gfa[:, col:col + 128],
                        rhs=wpk[:, wa_off:wa_off + width], start=True, stop=False,
                    )
                    nc.tensor.matmul(
                        pg[:, 0:width], lhsT=Gfb[0:68, col:col + 128],
                        rhs=wpk[0:68, wb_off:wb_off + width], start=False, stop=True,
                    )
                    o = j * FW + b * 64
                    if width == 128:
                        nc.scalar.activation(dst0[:, o:o + 64], pg[:, 0:64], fn)
                        nc.scalar.activation(dst1[:, o:o + 64], pg[:, 64:128], fn)
                    else:
                        nc.scalar.activation(dst0[:, o:o + 64], pg[:, 0:64], fn)

        # --- the 9 DCGRU cells ------------------------------------------------
        for t in range(T + 1):
            enc = t < T
            wa_ru = WCOL["eA_ru" if enc else "dA_ru"]
            wb_ru = WCOL["eB_ru" if enc else "dB_ru"]
            wa_c = WCOL["eA_c" if enc else "dA_c"]
            wb_c = WCOL["eB_c" if enc else "dB_c"]

            if t > 0:  # cell 0: h == 0, so Z1 = Z2 = 0 and Gfa/Gfb
                smatmul(hbf, z1bf)                 # Z1 = S h
                gfa_fill(hbf, z1bf)                # h | Z1 features
                smatmul(z1bf, z2bf)                # Z2 = S Z1
                gfb_fill(z2bf)                     # Z2 features
            # t == T: the GO symbol x = 0 comes from the zero rows of dxf
            for k in range(3):
                nc.sync.dma_start(out=Gfb[64 + k:65 + k, :],
                                  in_=dxf[k][t:t + 1, :])

            gates(wa_ru, wb_ru, 128, AF.Sigmoid, r_s, u_s)

            for j in range(NT):
                js = slice(j * FW, (j + 1) * FW)
                nc.vector.tensor_mul(r_s[:, js], r_s[:, js], h[:, js])  # rh
                nc.scalar.copy(rhbf[:, js], r_s[:, js])                 # rh bf16
            if t > 0:  # cell 0: rh = r*0 = 0, Z1' = Z2' = 0
                smatmul(rhbf, z1bf)                # Z1' = S rh
                gfa_fill(rhbf, z1bf)               # rh | Z1' features
                smatmul(z1bf, z2bf)                # Z2' = S Z1'
                gfb_fill(z2bf)

            gates(wa_c, wb_c, 64, AF.Tanh, c_s, None)

            for j in range(NT):
                js = slice(j * FW, (j + 1) * FW)
                tmp = tmpp.tile([128, FW], DT.float32, tag="tmp")
                nc.vector.tensor_sub(tmp[:], h[:, js], c_s[:, js])
                nc.vector.tensor_mul(tmp[:], tmp[:], u_s[:, js])
                nc.vector.tensor_add(h[:, js], c_s[:, js], tmp[:])
                nc.scalar.copy(hbf[:, js], h[:, js])

        # --- projection: out[b, m] = sum_u h * Wp + bp ------------------------
        for j in range(NT):
            js = slice(j * FW, (j + 1) * FW)
            tmp = tmpp.tile([128, FW], DT.float32, tag="tmp")
            nc.vector.tensor_mul(tmp[:], h[:, js], wp_sb[:])
            for b in range(BC):
                nc.vector.reduce_sum(
                    out_sb[:, j * BC + b:j * BC + b + 1],
                    tmp[:, b * 64:(b + 1) * 64],
                    axis=mybir.AxisListType.X,
                )
        for j in range(NT):
            nc.sync.dma_start(
                out=dOut[:, j * 128:(j + 1) * 128].rearrange("b p -> p b"),
                in_=out_sb[:, j * BC:(j + 1) * BC],
            )
    nc.finalize()
    return nc


_RUNNER = None  # (sharded_fn, in_names, out_names, out_shapes) — program-static
_RUNNER_MESH = None


def _get_runner():
    global _RUNNER
    if _RUNNER is not None:
        return _RUNNER

    import jax
    from jax.sharding import Mesh, PartitionSpec
    from jax.experimental.shard_map import shard_map
    from concourse.bass2jax import (
        _bass_exec_p, partition_id_tensor, install_neuronx_cc_hook,
    )

    nc = _build_program()
    install_neuronx_cc_hook()

    partition_name = (
        nc.partition_id_tensor.name if nc.partition_id_tensor else None
    )
    in_names, out_names, out_avals, out_shapes = [], [], [], []
    for alloc in nc.m.functions[0].allocations:
        if not isinstance(alloc, mybir.MemoryLocationSet):
            continue
        name = alloc.memorylocations[0].name
        if alloc.kind == "ExternalInput":
            if name != partition_name:
                in_names.append(name)
        elif alloc.kind == "ExternalOutput":
            shape = tuple(alloc.tensor_shape)
            dtype = mybir.dt.np(alloc.dtype)
            out_names.append(name)
            out_avals.append(jax.core.ShapedArray(shape, dtype))
            out_shapes.append((shape, dtype))
    n_params = len(in_names)
    n_outs = len(out_names)
    in_names_full = list(in_names) + out_names
    if partition_name is not None:
        in_names_full.append(partition_name)
    donate = tuple(range(n_params, n_params + n_outs))

    def _body(*args):
        operands = list(args)
        if partition_name is not None:
            operands.append(partition_id_tensor())
        outs = _bass_exec_p.bind(
            *operands,
            out_avals=tuple(out_avals),
            in_names=tuple(in_names_full),
            out_names=tuple(out_names),
            lowering_input_output_aliases=(),
            sim_require_finite=True,
            sim_require_nnan=True,
            nc=nc,
        )
        return tuple(outs)

    global _RUNNER_MESH
    devices = jax.devices()[:NCORES]
    mesh = Mesh(np.asarray(devices), ("core",))
    _RUNNER_MESH = mesh
    in_specs = (PartitionSpec("core"),) * (n_params + n_outs)
    out_specs = (PartitionSpec("core"),) * n_outs
    sharded = jax.jit(
        shard_map(_body, mesh=mesh, in_specs=in_specs, out_specs=out_specs,
                  check_rep=False),
        donate_argnums=donate,
        keep_unused=True,
    )
    _RUNNER = (sharded, in_names, out_names, out_shapes)
    return _RUNNER


def kernel(inputs, support, enc_W_ru, enc_b_ru, enc_W_c, enc_b_c,
           dec_W_ru, dec_b_ru, dec_W_c, dec_b_c, W_proj, b_proj):
    import ml_dtypes
    bf16 = ml_dtypes.bfloat16

    inputs = np.asarray(inputs, np.float32)
    support = np.asarray(support, np.float32)
    W_proj = np.asarray(W_proj, np.float32)
    b_proj = np.asarray(b_proj, np.float32)

    # gin: [S row shard | wpack column shard], gathered on device
    wpk_full = np.zeros((128, WPC), np.float32)
    for nm, (blkA, blkB) in (
        ("e_ru", _prep_gate(enc_W_ru, enc_b_ru)),
        ("e_c", _prep_gate(enc_W_c, enc_b_c)),
        ("d_ru", _prep_gate(dec_W_ru, dec_b_ru)),
        ("d_c", _prep_gate(dec_W_c, dec_b_c)),
    ):
        pre = nm[0]
        suf = nm[2:]
        oa = WCOL[f"{pre}A_{suf}"]
        ob = WCOL[f"{pre}B_{suf}"]
        wpk_full[:, oa:oa + blkA.shape[1]] = blkA
        wpk_full[0:68, ob:ob + blkB.shape[1]] = blkB
    wpk_full[:, WCOL["ident"]:WCOL["ident"] + 128] = np.eye(128)

    # persistent host staging buffers (every element below is rewritten per
    # call except deliberate zero padding, which no call ever dirties)
    global _HBUF
    if _HBUF is None:
        _HBUF = (
            np.empty((NCORES * 128, GINW + NT * T * BC), bf16),
            np.zeros((NPAD, NPAD), np.float32),
            np.zeros((128, NSLOT * 128), np.float32),
            np.zeros((B, T, NPAD), np.float32),
        )
    gx, S_pad, spack, xp = _HBUF
    S_pad[:N, :N] = support
    for s, (i, j) in enumerate(UPT):
        spack[:, s * 128:(s + 1) * 128] = S_pad[i * 128:(i + 1) * 128,
                                                j * 128:(j + 1) * 128]
    gx[:, :SSH] = (
        spack.astype(bf16).reshape(128, NCORES, SSH)
        .transpose(1, 0, 2).reshape(NCORES * 128, SSH)
    )
    gx[:, SSH:GINW] = (
        wpk_full.astype(bf16).reshape(128, NCORES, WSH)
        .transpose(1, 0, 2).reshape(NCORES * 128, WSH)
    )
    # node-major x: gx[c*128+p, GINW + j*64 + b*8 + t] = x[c*8+b, t, j*128+p]
    xp[:, :, :N] = inputs
    gx[:, GINW:] = (
        xp.reshape(NCORES, BC, T, NT, 128).transpose(0, 4, 3, 1, 2)
        .reshape(NCORES * 128, NT * T * BC).astype(bf16)
    )

    wpg = np.broadcast_to(
        np.tile(W_proj[:, 0].astype(np.float32), BC)[None, :], (NCORES, FW)
    )

    sharded, in_names, out_names, out_shapes = _get_runner()
    host = {"gx": gx, "wp": np.ascontiguousarray(wpg)}
    args = [host[nm] for nm in in_names]
    # The kernel writes every element of each output, so the donated output
    # canvases' contents are irrelevant: recycle the previous call's output
    # device buffers instead of uploading fresh zeros. The first call ships
    # zeros as committed device arrays so the jit signature never changes.
    global _PREV_OUTS
    if _PREV_OUTS is None:
        import jax
        from jax.sharding import NamedSharding, PartitionSpec
        mesh = _RUNNER_MESH
        sh = NamedSharding(mesh, PartitionSpec("core"))
        canvases = [
            jax.device_put(
                np.zeros((NCORES * shape[0], *shape[1:]), dtype), sh)
            for shape, dtype in out_shapes
        ]
    else:
        canvases = _PREV_OUTS
    outs = sharded(*args, *canvases)
    res = np.asarray(outs[out_names.index("out")])
    _PREV_OUTS = list(outs)
    return res[:, :N] + b_proj[0]


_PREV_OUTS = None
_HBUF = None


if __name__ == "__main__":
    pass


# revision 36
# speedup vs baseline: 1.1615x; 1.0631x over previous
"""DCRNN (DCGRU encoder x8 + decoder x1 + projection) on 8 TRN2 NeuronCores.

Sharding: data-parallel over batch (B=64 -> 8 per core). The support matrix S
(symmetric scaled Laplacian, padded 1000->1024) and the GRU weights are
row/column-sharded on the wire (2.3MB total instead of 8x replicated) and
rebuilt on every core with a single on-device AllGather over NeuronLink.

Per-core on-device algorithm, per DCGRU cell:
  Z1 = S @ h, Z2 = S @ Z1          (node-major [n,(b,u)] PE matmuls, bf16)
  ru = sigmoid(h@A + Z1@B + Z2@C + x-part + bias)   (feature-major gate
       matmuls fed by PE DMA-transposes)
  rh = r*h; Z1' = S@rh; Z2' = S@Z1'
  c  = tanh(...); h = u*h + (1-u)*c                 (DVE elementwise)
Chebyshev recurrence + the f*K+k torch weight layout are folded on the host
into per-part weight blocks:  out = h@A + Z1@B + Z2@C + x*wx0 + Sx*wx1
+ S2x*wx2 + bias, with [A;B] (128,out) and [C;wx;bias] (68,out) stacks.

Dispatch: one persistent jax.jit(shard_map(bass_exec)) callable built once
per process (rebuilding it per call costs >1s of retracing). Per call ~2.4MB
ride the axon tunnel: S upper-triangle tiles (the lower half is rebuilt by PE
transposes), a column-sharded weight pack (both all-gathered on device), and
per-core node-major x (Sx, S2x are computed on device). The donated output
canvases are the previous call's output buffers (every element is rewritten),
so no zero upload. Wall-clock is then ~2 tunnel round trips + bytes.
"""

import sys
import numpy as np

sys.path.insert(0, "/opt/trn_rl_repo")

from contextlib import ExitStack

import concourse.bass as bass
import concourse.bacc as bacc
import concourse.mybir as mybir
from concourse import tile

B, T, N, U = 64, 8, 1000, 64
NPAD = 1024
NCORES = 8
BC = B // NCORES          # 8 batch elements per core
NT = NPAD // 128          # 8 node tiles
FW = BC * U               # 512 free width: (b, u) b-major
DT = mybir.dt
AF = mybir.ActivationFunctionType

# packed weight SBUF tile [128, WPC]: column offsets of each block
WCOL = {
    "eA_ru": 0, "eB_ru": 128, "eA_c": 256, "eB_c": 320,
    "dA_ru": 384, "dB_ru": 512, "dA_c": 640, "dB_c": 704, "ident": 768,
}
WPC = 896
WSH = WPC // NCORES       # 112 wpack columns shipped per core
# S is symmetric: ship only upper-triangle 128x128 tiles (36, padded to 40
# slots so each core's column shard is 5 tile slots = 640 cols)
UPT = [(i, j) for i in range(NT) for j in range(i, NT)]
NSLOT = 40
SSH = NSLOT * 128 // NCORES   # 640 S-pack columns shipped per core
GINW = SSH + WSH              # 752: [S-triangle shard | wpack column shard]


def _prep_gate(W, b):
    """Fold Chebyshev recurrence + interleaved (f*K+k) weight layout into
    per-part blocks. out = x0@W0 + (S x0)@W1 + (2 S^2 x0 - x0)@W2 + b with
    x0 = [x | h]."""
    W = np.asarray(W, np.float32)
    b = np.asarray(b, np.float32)
    W0, W1, W2 = W[0::3], W[1::3], W[2::3]          # (65, out)
    A = W0[1:] - W2[1:]                             # h part
    Bh = W1[1:]                                     # Z1 part
    Ch = 2.0 * W2[1:]                               # Z2 part
    xrows = np.stack([W0[0] - W2[0], W1[0], 2.0 * W2[0]], 0)   # (3, out)
    blkA = np.concatenate([A, Bh], 0)               # (128, out)
    blkB = np.concatenate([Ch, xrows, b[None, :]], 0)  # (68, out)
    return blkA, blkB


def _build_program():
    nc = bacc.Bacc(None)

    # one bf16 input blob per core: [gather shard (S rows | wpack cols) | x]
    # x part is node-major: gx[p, GINW + j*64 + b*8 + t] = x[b, t, j*128+p]
    XW = NT * T * BC
    dGx = nc.declare_dram_parameter("gx", [128, GINW + XW], DT.bfloat16, False)
    dWp = nc.declare_dram_parameter("wp", [1, FW], DT.float32, False)
    dOut = nc.declare_dram_parameter("out", [BC, NPAD], DT.float32, True)

    with ExitStack() as ctx:
        tc = ctx.enter_context(tile.TileContext(nc))
        dram = ctx.enter_context(tc.tile_pool(name="dram", bufs=1, space="DRAM"))
        const = ctx.enter_context(tc.tile_pool(name="const", bufs=1))
        state = ctx.enter_context(tc.tile_pool(name="state", bufs=1))
        psS = ctx.enter_context(tc.tile_pool(name="psS", bufs=2, space="PSUM"))
        psG = ctx.enter_context(tc.tile_pool(name="psG", bufs=2, space="PSUM"))
        psT = ctx.enter_context(tc.tile_pool(name="psT", bufs=4, space="PSUM"))
        tmpp = ctx.enter_context(tc.tile_pool(name="tmpp", bufs=3))

        # --- gather S + packed weights from all cores -------------------------
        g_in_b = dram.tile([128, GINW], DT.bfloat16, tag="g_in_b")
        g_out = dram.tile([NCORES * 128, GINW], DT.bfloat16, tag="g_out",
                          addr_space="Shared")
        nc.gpsimd.dma_start(out=g_in_b[:], in_=dGx[:, 0:GINW])
        nc.gpsimd.collective_compute(
            "AllGather",
            mybir.AluOpType.bypass,
            replica_groups=[list(range(NCORES))],
            ins=[g_in_b.opt()],
            outs=[g_out.opt()],
        )

        # --- resident tensors -------------------------------------------------
        # S_sb[p, (i*NT+j)*128+q] = S_pad[i*128+p, j*128+q]; upper tiles come
        # from the gather, lower tiles by PE-transposing their mirror (S = S^T)
        wpk = const.tile([128, WPC], DT.bfloat16, tag="wpk")
        for c in range(NCORES):
            nc.sync.dma_start(
                out=wpk[:, c * WSH:(c + 1) * WSH],
                in_=g_out[c * 128:(c + 1) * 128, SSH:GINW],
            )
        ident = wpk[:, WCOL["ident"]:WCOL["ident"] + 128]

        S_sb = const.tile([128, NT * NT * 128], DT.bfloat16, tag="S_sb")
        for s, (i, j) in enumerate(UPT):
            nc.sync.dma_start(
                out=S_sb[:, (i * NT + j) * 128:(i * NT + j + 1) * 128],
                in_=g_out[(s // 5) * 128:(s // 5 + 1) * 128,
                          (s % 5) * 128:(s % 5 + 1) * 128],
            )
        for i, j in UPT:
            if i == j:
                continue
            pt = psT.tile([128, 128], DT.bfloat16, tag="pt")
            nc.tensor.transpose(
                pt[:], S_sb[:, (i * NT + j) * 128:(i * NT + j + 1) * 128],
                ident)
            nc.scalar.copy(
                S_sb[:, (j * NT + i) * 128:(j * NT + i + 1) * 128], pt[:])

        # broadcast wp row to all 128 partitions via a rank-1 PE outer product
        wp_row = const.tile([1, FW], DT.float32, tag="wp_row")
        ones_t = const.tile([1, 128], DT.float32, tag="ones_t")
        wp_sb = const.tile([128, FW], DT.float32, tag="wp_sb")
        nc.sync.dma_start(out=wp_row[0:1, :], in_=dWp[:])
        nc.vector.memset(ones_t[0:1, :], 1.0)
        ps_w = psS.tile([128, FW], DT.float32, tag="psS")
        nc.tensor.matmul(ps_w[:], lhsT=ones_t[0:1, :], rhs=wp_row[0:1, :],
                         start=True, stop=True)
        nc.vector.tensor_copy(wp_sb[:], ps_w[:])

        Gfa = state.tile([128, BC * NPAD], DT.bfloat16, tag="Gfa")
        Gfb = state.tile([128, BC * NPAD], DT.bfloat16, tag="Gfb")
        h = state.tile([128, NT * FW], DT.float32, tag="h")
        hbf = state.tile([128, NT * FW], DT.bfloat16, tag="hbf")
        z1bf = state.tile([128, NT * FW], DT.bfloat16, tag="z1bf")
        z2bf = state.tile([128, NT * FW], DT.bfloat16, tag="z2bf")
        rhbf = state.tile([128, NT * FW], DT.bfloat16, tag="rhbf")
        r_s = state.tile([128, NT * FW], DT.float32, tag="r_s")   # r, then rh
        u_s = state.tile([128, NT * FW], DT.float32, tag="u_s")
        c_s = state.tile([128, NT * FW], DT.float32, tag="c_s")
        out_sb = state.tile([128, NT * BC], DT.float32, tag="out_sb")

        nc.vector.memset(h[:], 0.0)
        nc.vector.memset(hbf[:], 0.0)
        nc.vector.memset(Gfa[:], 0.0)
        nc.vector.memset(Gfb[0:64, :], 0.0)

        def gfa_fill(src0_bf, src1_bf):
            # PE-transpose src0 (rows 0:64) + src1 (rows 64:128) per (j,b)
            # into one PSUM tile, one ACT copy out to Gfa.
            for j in range(NT):
                for b in range(BC):
                    pt = psT.tile([128, 128], DT.bfloat16, tag="pt")
                    s = slice(j * FW + b * 64, j * FW + (b + 1) * 64)
                    nc.tensor.transpose(pt[0:64, :], src0_bf[:, s], ident)
                    nc.tensor.transpose(pt[64:128, :], src1_bf[:, s], ident)
                    col = b * NPAD + j * 128
                    nc.scalar.copy(Gfa[:, col:col + 128], pt[:])

        def gfb_fill(src_bf):
            for j in range(NT):
                for b in range(BC):
                    pt = psT.tile([128, 128], DT.bfloat16, tag="pt")
                    s = slice(j * FW + b * 64, j * FW + (b + 1) * 64)
                    nc.tensor.transpose(pt[0:64, :], src_bf[:, s], ident)
                    col = b * NPAD + j * 128
                    nc.scalar.copy(Gfb[0:64, col:col + 128], pt[0:64, :])

        def smatmul(rhs_bf, out_bf):
            # Z = S @ rhs  (node-major in/out), bf16 on PE, fp32 accum
            for j in range(NT):
                ps = psS.tile([128, FW], DT.float32, tag="psS")
                for i in range(NT):
                    nc.tensor.matmul(
                        ps[:],
                        lhsT=S_sb[:, (i * NT + j) * 128:(i * NT + j + 1) * 128],
                        rhs=rhs_bf[:, i * FW:(i + 1) * FW],
                        start=(i == 0),
                        stop=(i == NT - 1),
                    )
                nc.vector.tensor_copy(out_bf[:, j * FW:(j + 1) * FW], ps[:])

        # --- x features: x, Sx, S2x computed on device ------------------------
        # node-major x tiles [128, j*64 + t*8 + b], diffused by S on the PE,
        # then DMA-scattered into DRAM in feature-major per-cell row layout
        # dxf_k[t, b*NPAD + j*128 + p]  (rows T: zeros for GO, T+1 on k=0: ones)
        FWX = T * BC
        x0_nm = state.tile([128, NT * FWX], DT.bfloat16, tag="x0_nm")
        x1_nm = state.tile([128, NT * FWX], DT.bfloat16, tag="x1_nm")
        x2_nm = state.tile([128, NT * FWX], DT.bfloat16, tag="x2_nm")
        nc.sync.dma_start(out=x0_nm[:], in_=dGx[:, GINW:GINW + XW])

        def smatmul_x(rhs_bf, out_bf):
            for j in range(NT):
                ps = psS.tile([128, FW], DT.float32, tag="psS")
                for i in range(NT):
                    nc.tensor.matmul(
                        ps[:, 0:FWX],
                        lhsT=S_sb[:, (i * NT + j) * 128:(i * NT + j + 1) * 128],
                        rhs=rhs_bf[:, i * FWX:(i + 1) * FWX],
                        start=(i == 0),
                        stop=(i == NT - 1),
                    )
                nc.vector.tensor_copy(out_bf[:, j * FWX:(j + 1) * FWX],
                                      ps[:, 0:FWX])

        smatmul_x(x0_nm, x1_nm)
        smatmul_x(x1_nm, x2_nm)

        zrow = const.tile([1, BC * NPAD], DT.bfloat16, tag="zrow")
        orow = const.tile([1, BC * NPAD], DT.bfloat16, tag="orow")
        nc.vector.memset(zrow[0:1, :], 0.0)
        nc.vector.memset(orow[0:1, :], 1.0)
        dxf = []
        for k, src in ((0, x0_nm), (1, x1_nm), (2, x2_nm)):
            dx = dram.tile([T + 2, BC * NPAD], DT.bfloat16, tag=f"dxf{k}",
                           name=f"dxf{k}")
            for j in range(NT):
                # [128 nodes, (b t)] -> PE transpose -> [(b t), 128 nodes]
                pt = psT.tile([128, 128], DT.bfloat16, tag="pt")
                nc.tensor.transpose(pt[0:64, :],
                                    src[:, j * FWX:(j + 1) * FWX], ident)
                xt = tmpp.tile([128, 128], DT.bfloat16, tag="xt")
                nc.scalar.copy(xt[0:64, :], pt[0:64, :])
                for b in range(BC):
                    o = b * NPAD + j * 128
                    nc.sync.dma_start(out=dx[0:T, o:o + 128],
                                      in_=xt[b * T:(b + 1) * T, :])
            nc.sync.dma_start(out=dx[T:T + 1, :], in_=zrow[0:1, :])
            dxf.append(dx)
        nc.sync.dma_start(out=dxf[0][T + 1:T + 2, :], in_=orow[0:1, :])
        nc.sync.dma_start(out=Gfb[67:68, :], in_=dxf[0][T + 1:T + 2, :])

        def gates(wa_off, wb_off, width, fn, dst0, dst1):
            # psum[m,out] = Gfa_slice.T @ wa + Gfb_slice.T @ wb ; act -> dst
            for j in range(NT):
                for b in range(BC):
                    pg = psG.tile([128, 128], DT.float32, tag="psG")
                    col = b * NPAD + j * 128
                    nc.tensor.matmul(
                        pg[:, 0:width], lhsT=Gfa[:, col:col + 128],
                        rhs=wpk[:, wa_off:wa_off + width], start=True, stop=False,
                    )
                    nc.tensor.matmul(
                        pg[:, 0:width], lhsT=Gfb[0:68, col:col + 128],
                        rhs=wpk[0:68, wb_off:wb_off + width], start=False, stop=True,
                    )
                    o = j * FW + b * 64
                    if width == 128:
                        nc.scalar.activation(dst0[:, o:o + 64], pg[:, 0:64], fn)
                        nc.scalar.activation(dst1[:, o:o + 64], pg[:, 64:128], fn)
                    else:
                        nc.scalar.activation(dst0[:, o:o + 64], pg[:, 0:64], fn)

        # --- the 9 DCGRU cells ------------------------------------------------
        for t in range(T + 1):
            enc = t < T
            wa_ru = WCOL["eA_ru" if enc else "dA_ru"]
            wb_ru = WCOL["eB_ru" if enc else "dB_ru"]
            wa_c = WCOL["eA_c" if enc else "dA_c"]
            wb_c = WCOL["eB_c" if enc else "dB_c"]

            if t > 0:  # cell 0: h == 0, so Z1 = Z2 = 0 and Gfa/Gfb
                smatmul(hbf, z1bf)                 # Z1 = S h
                gfa_fill(hbf, z1bf)                # h | Z1 features
                smatmul(z1bf, z2bf)                # Z2 = S Z1
                gfb_fill(z2bf)                     # Z2 features
            # t == T: the GO symbol x = 0 comes from the zero rows of dxf
            for k in range(3):
                nc.sync.dma_start(out=Gfb[64 + k:65 + k, :],
                                  in_=dxf[k][t:t + 1, :])

            gates(wa_ru, wb_ru, 128, AF.Sigmoid, r_s, u_s)

            for j in range(NT):
                js = slice(j * FW, (j + 1) * FW)
                nc.vector.tensor_mul(r_s[:, js], r_s[:, js], h[:, js])  # rh
                nc.scalar.copy(rhbf[:, js], r_s[:, js])                 # rh bf16
            if t > 0:  # cell 0: rh = r*0 = 0, Z1' = Z2' = 0
                smatmul(rhbf, z1bf)                # Z1' = S rh
                gfa_fill(rhbf, z1bf)               # rh | Z1' features
                smatmul(z1bf, z2bf)                # Z2' = S Z1'
                gfb_fill(z2bf)

            gates(wa_c, wb_c, 64, AF.Tanh, c_s, None)

            for j in range(NT):
                js = slice(j * FW, (j + 1) * FW)
                tmp = tmpp.tile([128, FW], DT.float32, tag="tmp")
                nc.vector.tensor_sub(tmp[:], h[:, js], c_s[:, js])
                nc.vector.tensor_mul(tmp[:], tmp[:], u_s[:, js])
                nc.vector.tensor_add(h[:, js], c_s[:, js], tmp[:])
                nc.scalar.copy(hbf[:, js], h[:, js])

        # --- projection: out[b, m] = sum_u h * Wp + bp ------------------------
        for j in range(NT):
            js = slice(j * FW, (j + 1) * FW)
            tmp = tmpp.tile([128, FW], DT.float32, tag="tmp")
            nc.vector.tensor_mul(tmp[:], h[:, js], wp_sb[:])
            for b in range(BC):
                nc.vector.reduce_sum(
                    out_sb[:, j * BC + b:j * BC + b + 1],
                    tmp[:, b * 64:(b + 1) * 64],
                    axis=mybir.AxisListType.X,
                )
        for j in range(NT):
            nc.sync.dma_start(
                out=dOut[:, j * 128:(j + 1) * 128].rearrange("b p -> p b"),
                in_=out_sb[:, j * BC:(j + 1) * BC],
            )
    nc.finalize()
    return nc


_RUNNER = None  # (sharded_fn, in_names, out_names, out_shapes) — program-static
_RUNNER_MESH = None


def _get_runner():
    global _RUNNER
    if _RUNNER is not None:
        return _RUNNER

    import jax
    from jax.sharding import Mesh, PartitionSpec
    from jax.experimental.shard_map import shard_map
    from concourse.bass2jax import (
        _bass_exec_p, partition_id_tensor, install_neuronx_cc_hook,
    )

    nc = _build_program()
    install_neuronx_cc_hook()

    partition_name = (
        nc.partition_id_tensor.name if nc.partition_id_tensor else None
    )
    in_names, out_names, out_avals, out_shapes = [], [], [], []
    for alloc in nc.m.functions[0].allocations:
        if not isinstance(alloc, mybir.MemoryLocationSet):
            continue
        name = alloc.memorylocations[0].name
        if alloc.kind == "ExternalInput":
            if name != partition_name:
                in_names.append(name)
        elif alloc.kind == "ExternalOutput":
            shape = tuple(alloc.tensor_shape)
            dtype = mybir.dt.np(alloc.dtype)
            out_names.append(name)
            out_avals.append(jax.core.ShapedArray(shape, dtype))
            out_shapes.append((shape, dtype))
    n_params = len(in_names)
    n_outs = len(out_names)
    in_names_full = list(in_names) + out_names
    if partition_name is not None:
        in_names_full.append(partition_name)
    donate = tuple(range(n_params, n_params + n_outs))

    def _body(*args):
        operands = list(args)
        if partition_name is not None:
            operands.append(partition_id_tensor())
        outs = _bass_exec_p.bind(
            *operands,
            out_avals=tuple(out_avals),
            in_names=tuple(in_names_full),
            out_names=tuple(out_names),
            lowering_input_output_aliases=(),
            sim_require_finite=True,
            sim_require_nnan=True,
            nc=nc,
        )
        return tuple(outs)

    global _RUNNER_MESH
    devices = jax.devices()[:NCORES]
    mesh = Mesh(np.asarray(devices), ("core",))
    _RUNNER_MESH = mesh
    in_specs = (PartitionSpec("core"),) * (n_params + n_outs)
    out_specs = (PartitionSpec("core"),) * n_outs
    sharded = jax.jit(
        shard_map(_body, mesh=mesh, in_specs=in_specs, out_specs=out_specs,
                  check_rep=False),
        donate_argnums=donate,
        keep_unused=True,
    )
    _RUNNER = (sharded, in_names, out_names, out_shapes)
    return _RUNNER


def kernel(inputs, support, enc_W_ru, enc_b_ru, enc_W_c, enc_b_c,
           dec_W_ru, dec_b_ru, dec_W_c, dec_b_c, W_proj, b_proj):
    import ml_dtypes
    bf16 = ml_dtypes.bfloat16

    inputs = np.asarray(inputs, np.float32)
    support = np.asarray(support, np.float32)
    W_proj = np.asarray(W_proj, np.float32)
    b_proj = np.asarray(b_proj, np.float32)

    # gin: [S row shard | wpack column shard], gathered on device
    wpk_full = np.zeros((128, WPC), np.float32)
    for nm, (blkA, blkB) in (
        ("e_ru", _prep_gate(enc_W_ru, enc_b_ru)),
        ("e_c", _prep_gate(enc_W_c, enc_b_c)),
        ("d_ru", _prep_gate(dec_W_ru, dec_b_ru)),
        ("d_c", _prep_gate(dec_W_c, dec_b_c)),
    ):
        pre = nm[0]
        suf = nm[2:]
        oa = WCOL[f"{pre}A_{suf}"]
        ob = WCOL[f"{pre}B_{suf}"]
        wpk_full[:, oa:oa + blkA.shape[1]] = blkA
        wpk_full[0:68, ob:ob + blkB.shape[1]] = blkB
    wpk_full[:, WCOL["ident"]:WCOL["ident"] + 128] = np.eye(128)

    # persistent host staging buffers (every element below is rewritten per
    # call except deliberate zero padding, which no call ever dirties)
    global _HBUF
    if _HBUF is None:
        _HBUF = (
            np.empty((NCORES * 128, GINW + NT * T * BC), bf16),
            np.zeros((NPAD, NPAD), np.float32),
            np.zeros((128, NSLOT * 128), np.float32),
            np.zeros((B, T, NPAD), np.float32),
        )
    gx, S_pad, spack, xp = _HBUF
    S_pad[:N, :N] = support
    for s, (i, j) in enumerate(UPT):
        spack[:, s * 128:(s + 1) * 128] = S_pad[i * 128:(i + 1) * 128,
                                                j * 128:(j + 1) * 128]
    gx[:, :SSH] = (
        spack.astype(bf16).reshape(128, NCORES, SSH)
        .transpose(1, 0, 2).reshape(NCORES * 128, SSH)
    )
    gx[:, SSH:GINW] = (
        wpk_full.astype(bf16).reshape(128, NCORES, WSH)
        .transpose(1, 0, 2).reshape(NCORES * 128, WSH)
    )
    # node-major x: gx[c*128+p, GINW + j*64 + b*8 + t] = x[c*8+b, t, j*128+p]
    xp[:, :, :N] = inputs
    gx[:, GINW:] = (
        xp.reshape(NCORES, BC, T, NT, 128).transpose(0, 4, 3, 1, 2)
        .reshape(NCORES * 128, NT * T * BC).astype(bf16)
    )

    wpg = np.broadcast_to(
        np.tile(W_proj[:, 0].astype(np.float32), BC)[None, :], (NCORES, FW)
    )

    sharded, in_names, out_names, out_shapes = _get_runner()
    host = {"gx": gx, "wp": np.ascontiguousarray(wpg)}
    args = [host[nm] for nm in in_names]
    # The kernel writes every element of each output, so the donated output
    # canvases' contents are irrelevant: recycle the previous call's output
    # device buffers instead of uploading fresh zeros. The first call ships
    # zeros as committed device arrays so the jit signature never changes.
    global _PREV_OUTS

    def _fresh_canvases():
        import jax
        from jax.sharding import NamedSharding, PartitionSpec
        sh = NamedSharding(_RUNNER_MESH, PartitionSpec("core"))
        return [
            jax.device_put(
                np.zeros((NCORES * shape[0], *shape[1:]), dtype), sh)
            for shape, dtype in out_shapes
        ]

    canvases = _PREV_OUTS if _PREV_OUTS is not None else _fresh_canvases()
    try:
        outs = sharded(*args, *canvases)
        res = np.asarray(outs[out_names.index("out")])
    except Exception:
        _PREV_OUTS = None
        outs = sharded(*args, *_fresh_canvases())
        res = np.asarray(outs[out_names.index("out")])
    _PREV_OUTS = list(outs)
    return res[:, :N] + b_proj[0]


_PREV_OUTS = None
_HBUF = None


if __name__ == "__main__":
    pass
